# revision 3
# baseline (speedup 1.0000x reference)
"""Trainium2 Bass kernel for nn_DualAttention (S=2048, B=16, H2=2048, V=1024).

Computation (per the reference):
    sum_w = hidden @ Ww + bw + z @ Wz + bz + w_a*0.5        [S, B, V]
    u     = tanh(sum_w) @ Vw + vb                            [S, B, 1]
    out   = softmax(u, axis=0)                               [S, B, 1]

Strategy
--------
Data-parallel over batch: 16 batches -> 2 per NeuronCore (8 cores).
Host-side prep per core:
  * concat hidden/z along the hidden axis -> X [ROWS=4096, H=4096]
    (rows are b-major: row = b_local*2048 + s)
  * transpose to xt = X^T [H, ROWS] and cast to bf16 (matmul dtype)
  * W = concat([Ww, Wz], 0) [H, V] bf16;  bias = bw + bz + 0.5*w_a
Device kernel (per core), W-stationary matmul with psum layout [v, rows]:
  for each rowblock (512 rows):
    load xt[:, rowblock] into SBUF ([128, 32k, 512])
    for vb in 0..7:                       # 128-wide slices of V
      psum[vb] += sum_k W[k,vb].T @ xt[k]      (32 accumulating matmuls)
      t = tanh(psum + bias_vb)            # one ACT op, per-partition bias
      u_psum += Vw[vb].T @ t              # [1, 512] second-stage matmul (f32r)
    u_all[0, rowblock] = u_psum
  softmax over s per batch (no max subtraction: u is tanh-bounded):
    reshape u_all [1,4096]->[2,2048] via DMA, exp+rowsum on ACT,
    reciprocal + scale on DVE, DMA out [2, 2048].

The vb scalar is dropped: softmax is shift-invariant.
"""

import sys
import types

import numpy as np
import ml_dtypes

# ---------------------------------------------------------------------------
# Problem constants (hardcoded; kernel.py must be self-contained)
# ---------------------------------------------------------------------------
S, B, H2, V = 2048, 16, 2048, 1024
ALPHA_S = 0.5
NCORES = 8
BC = B // NCORES            # local batches per core
ROWS = S * BC               # 4096 rows per core (b-major)
H = 2 * H2                  # 4096 contraction dim (hidden ++ z)
P = 128
RB = 512                    # rows per block
NRB = ROWS // RB            # 8
NK = H // P                 # 32
NVB = V // P                # 8

MAIN_DT = "bf16"            # main matmul dtype: "bf16" | "f32r"


# ---------------------------------------------------------------------------
# Workarounds for this walrus build's 1-sync-wait-per-instruction limit
# ---------------------------------------------------------------------------
def _install_drain_patch():
    import concourse.mybir as mybir
    from concourse.tile import TileContext
    from concourse.vector_clock import ScopedClock

    def _drain_and_barrier(self, tick_clock, wait_clock):
        nc = self.nc
        drain_inst = nc.sync.drain()
        wait_clock.add_sem_waits(
            drain_inst.ins, ScopedClock({None: tick_clock.global_clock})
        )
        si = drain_inst.ins.sync_info
        if si is not None:
            waits = list(si.on_wait)
            if len(waits) > 1:
                si.on_wait = [waits[0]]
                for w in waits[1:]:
                    nop = nc.sync.nop(nofuse=True)
                    nop.ins.sync_info = mybir.SyncInfo(on_wait=[w], on_update=[])
        nc.all_engine_barrier()
        assert self.sems is not None
        popped = nc._tile_sem_poison_stack.pop()
        assert popped is self._sem_poison
        nc.clear_and_free_semaphores(list(self.sems.allocated().values()))
        nc.all_engine_barrier()

    TileContext._drain_and_barrier = _drain_and_barrier


def _split_multiwait(nc):
    """Hoist extra sync waits onto same-engine event-semaphore instructions
    inserted just before the carrying instruction."""
    import concourse.mybir as mybir

    counter = 0
    for fn in nc.m.functions:
        for bb in fn.blocks:
            insts = bb.instructions
            new_list = []
            changed = False
            for inst in insts:
                si = inst.sync_info
                if si is not None:
                    waits = list(si.on_wait)
                    if len(waits) > 1:
                        for w in waits[:-1]:
                            counter += 1
                            nop = mybir.InstEventSemaphore(
                                name=f"I-mwsplit-{counter}"
                            )
                            nop.engine = inst.engine
                            nop.bass_nofuse = True
                            nop.sync_info = mybir.SyncInfo(
                                on_wait=[w], on_update=[]
                            )
                            nc.register_instruction(nop)
                            new_list.append(nop)
                        si.on_wait = [waits[-1]]
                        changed = True
                new_list.append(inst)
            if changed:
                bb.instructions = new_list
    return counter


# ---------------------------------------------------------------------------
# Kernel build
# ---------------------------------------------------------------------------
def _build_nc():
    import concourse.bass as bass
    import concourse.mybir as mybir
    from concourse.tile import TileContext

    f32 = mybir.dt.float32
    f32r = mybir.dt.float32r
    if MAIN_DT == "bf16":
        DT = mybir.dt.bfloat16
    else:
        DT = f32r

    nc = bass.Bass()
    xt_d = nc.declare_dram_parameter("xt", [H, ROWS], DT, isOutput=False)
    w_d = nc.declare_dram_parameter("w", [H, V], DT, isOutput=False)
    bct_d = nc.declare_dram_parameter("bct", [P, NVB], f32, isOutput=False)
    vwt_d = nc.declare_dram_parameter("vwt", [P, NVB], f32r, isOutput=False)
    att_d = nc.declare_dram_parameter("att", [BC, S], f32, isOutput=True)

    with TileContext(nc) as tc:
        with (
            tc.tile_pool(name="wpool", bufs=1) as wpool,
            tc.tile_pool(name="xpool", bufs=2) as xpool,
            tc.tile_pool(name="tpool", bufs=3) as tpool,
            tc.tile_pool(name="spool", bufs=1) as spool,
            tc.tile_pool(name="pspool", bufs=2, space="PSUM") as pspool,
            tc.tile_pool(name="upspool", bufs=2, space="PSUM") as upspool,
        ):
            # --- resident weights / constants ---
            w_sb = wpool.tile([P, NK, V], DT, name="w_sb")
            # DRAM W [H, V]: partition p<-h within k-tile, free (k, v)
            nc.sync.dma_start(
                out=w_sb[:], in_=w_d[:, :].rearrange("(k p) v -> p k v", p=P)
            )
            bct_sb = spool.tile([P, NVB], f32, name="bct_sb")
            nc.sync.dma_start(out=bct_sb[:], in_=bct_d[:, :])
            vwt_sb = spool.tile([P, NVB], f32r, name="vwt_sb")
            nc.sync.dma_start(out=vwt_sb[:], in_=vwt_d[:, :])

            u_all = spool.tile([1, ROWS], f32, name="u_all")

            xt_r = xt_d[:, :].rearrange("(k p) (r c) -> p r k c", p=P, c=RB)

            for r in range(NRB):
                xt_sb = xpool.tile([P, NK, RB], DT, name="xt_sb", tag="xt")
                nc.sync.dma_start(out=xt_sb[:], in_=xt_r[:, r])

                u_ps = upspool.tile([1, RB], f32, name="u_ps", tag="ups")
                for vb in range(NVB):
                    ps = pspool.tile([P, RB], f32, name="ps", tag="ps")
                    for k in range(NK):
                        nc.tensor.matmul(
                            ps[:],
                            w_sb[:, k, vb * P : (vb + 1) * P],
                            xt_sb[:, k],
                            start=(k == 0),
                            stop=(k == NK - 1),
                        )
                    tt = tpool.tile([P, RB], f32r, name="tt", tag="tt")
                    nc.scalar.activation(
                        tt[:],
                        ps[:],
                        mybir.ActivationFunctionType.Tanh,
                        bias=bct_sb[:, vb : vb + 1],
                        scale=1.0,
                    )
                    nc.tensor.matmul(
                        u_ps[:],
                        vwt_sb[:, vb : vb + 1],
                        tt[:],
                        start=(vb == 0),
                        stop=(vb == NVB - 1),
                    )
                nc.vector.tensor_copy(u_all[:, r * RB : (r + 1) * RB], u_ps[:])

            # --- softmax over s per local batch ---
            # reshape [1, 4096] -> [2, 2048] (rows are b-major)
            u2 = spool.tile([BC, S], f32, name="u2")
            nc.sync.dma_start(
                out=u2[:], in_=u_all[:, :].rearrange("o (b s) -> o b s", b=BC)
            )
            ex = spool.tile([BC, S], f32, name="ex")
            esum = spool.tile([BC, 1], f32, name="esum")
            nc.scalar.activation(
                ex[:],
                u2[:],
                mybir.ActivationFunctionType.Exp,
                accum_out=esum[:],
            )
            rec = spool.tile([BC, 1], f32, name="rec")
            nc.vector.reciprocal(rec[:], esum[:])
            att_sb = spool.tile([BC, S], f32, name="att_sb")
            nc.vector.tensor_scalar_mul(att_sb[:], ex[:], rec[:])
            nc.sync.dma_start(out=att_d[:, :], in_=att_sb[:])

    _split_multiwait(nc)
    return nc


# ---------------------------------------------------------------------------
# Host entry point
# ---------------------------------------------------------------------------
def kernel(hidden, z, Ww, bw, Wz, bz, Vw, vb, w_a):
    _install_drain_patch()
    from concourse.bass_utils import run_bass_kernel_spmd

    np_main = ml_dtypes.bfloat16 if MAIN_DT == "bf16" else np.float32

    # ---- host-side shard prep ----
    # xt[core] = concat(hidden_slice, z_slice along h).T  -> [H, ROWS]
    hid_t = np.ascontiguousarray(
        np.asarray(hidden).astype(np_main).transpose(2, 1, 0)
    )  # [H2, B, S]
    z_t = np.ascontiguousarray(
        np.asarray(z).astype(np_main).transpose(2, 1, 0)
    )  # [H2, B, S]

    w_cat = np.concatenate(
        [np.asarray(Ww), np.asarray(Wz)], axis=0
    ).astype(np_main)  # [H, V]

    bias = (
        np.asarray(bw).astype(np.float64)
        + np.asarray(bz).astype(np.float64)
        + float(np.asarray(w_a)) * ALPHA_S
    ).astype(np.float32)  # [V]
    bct = np.ascontiguousarray(bias.reshape(NVB, P).T)  # [P, NVB]
    vwt = np.ascontiguousarray(
        np.asarray(Vw).astype(np.float32).reshape(NVB, P).T
    )  # [P, NVB]

    in_maps = []
    for c in range(NCORES):
        xt_c = np.empty((H, ROWS), dtype=np_main)
        # rows b-major: row = b_local*S + s
        xt_c[:H2] = hid_t[:, 2 * c : 2 * c + 2, :].reshape(H2, ROWS)
        xt_c[H2:] = z_t[:, 2 * c : 2 * c + 2, :].reshape(H2, ROWS)
        in_maps.append({"xt": xt_c, "w": w_cat, "bct": bct, "vwt": vwt})

    nc = _build_nc()
    res = run_bass_kernel_spmd(nc, in_maps, list(range(NCORES)))

    out = np.empty((S, B, 1), dtype=np.float32)
    for c in range(NCORES):
        att = res.results[c]["att"]  # [BC, S]
        for b in range(BC):
            out[:, 2 * c + b, 0] = att[b]
    return out
